# revision 1
# baseline (speedup 1.0000x reference)
"""AttnDecoderRNN Trainium2 kernel.

B=128 batch data-parallel over 8 cores (16/core). Per core:
  pre-loop: U[b] = Ua @ enc[b]^T   (layout [g(part), t]), enc resident as [t(part), h]
  per step: qT = Wa h  -> X = tanh(U + q) (ACT, per-partition bias)
            scores = va^T X (M=1 matmuls packed 4x into PE col-groups)
            softmax -> w -> wT (PE transpose) -> ctx = w^T enc (col-group M=1)
            gates = ctxT Wctx + hT Whh + Gc (PSUM accum, I16 trick for Gc)
            LSTM cell via tanh-only (sigmoid(x) = (1+tanh(x/2))/2, 0.5 folded
            into Wa/Whh/Wp host-side; h stored as 2h)
            yT = Wp h + bp -> DMA out per step.
"""

import numpy as np
import ml_dtypes
from contextlib import ExitStack

import concourse.bass as bass
import concourse.tile as tile
from concourse import bacc, mybir
from concourse.bass_utils import run_bass_kernel_spmd

F32 = mybir.dt.float32
BF16 = mybir.dt.bfloat16
AF = mybir.ActivationFunctionType
ALU = mybir.AluOpType
AX = mybir.AxisListType

B, T, H, D = 128, 512, 512, 128
NCORES = 8
BL = B // NCORES  # 16
HC = H // 128     # 4 h-chunks
TC = T // 128     # 4 t-chunks
G4 = 4 * H        # 2048


def build(out_len: int, unroll: bool = False, bench_steps=None) -> bass.Bass:
    nc = bacc.Bacc(None, target_bir_lowering=False)

    encT = nc.dram_tensor("encT", [BL, HC, 128, T], BF16, kind="ExternalInput")
    enct = nc.dram_tensor("enct", [BL, TC, 128, H], BF16, kind="ExternalInput")
    uaT = nc.dram_tensor("uaT", [HC, 128, H], BF16, kind="ExternalInput")
    waT = nc.dram_tensor("waT", [HC, 128, H], BF16, kind="ExternalInput")
    wctxT = nc.dram_tensor("wctxT", [HC, 128, G4], BF16, kind="ExternalInput")
    whhT = nc.dram_tensor("whhT", [HC, 128, G4], BF16, kind="ExternalInput")
    wpT = nc.dram_tensor("wpT", [HC, 128, D], BF16, kind="ExternalInput")
    vaT = nc.dram_tensor("vaT", [128, HC], BF16, kind="ExternalInput")
    gcw = nc.dram_tensor("gcw", [BL, G4], BF16, kind="ExternalInput")
    bpw = nc.dram_tensor("bpw", [128, 1], F32, kind="ExternalInput")
    id16 = nc.dram_tensor("id16", [16, 16], F32, kind="ExternalInput")
    i16b = nc.dram_tensor("i16b", [16, 16], BF16, kind="ExternalInput")
    yT = nc.dram_tensor("yT", [out_len, 128, BL], F32, kind="ExternalOutput")

    with tile.TileContext(nc) as tc, ExitStack() as ctx:
        singles = ctx.enter_context(tc.tile_pool(name="singles", bufs=1))
        # --- resident SBUF tensors ---
        U_sb = singles.tile([128, BL, HC, T], BF16)       # 64KB/part
        enct_sb = singles.tile([128, BL, TC, H], BF16)    # 64KB/part
        waT_sb = singles.tile([128, HC, H], BF16)
        wctxT_sb = singles.tile([128, HC, G4], BF16)
        whhT_sb = singles.tile([128, HC, G4], BF16)
        wpT_sb = singles.tile([128, HC, D], BF16)
        vaT_sb = singles.tile([128, HC], BF16)
        gc_sb = singles.tile([BL, G4], BF16)
        bp_sb = singles.tile([128, 1], F32)
        id16_sb = singles.tile([16, 16], F32)
        i16b_sb = singles.tile([16, 16], BF16)
        hsT_sb = singles.tile([128, HC, BL], BF16)        # 2h, transposed
        cs_sb = singles.tile([BL, H], F32)                # true c

        nc.gpsimd.dma_start(out=waT_sb[:], in_=waT.rearrange("k p t -> p k t"))
        nc.gpsimd.dma_start(out=wctxT_sb[:], in_=wctxT.rearrange("k p t -> p k t"))
        nc.gpsimd.dma_start(out=whhT_sb[:], in_=whhT.rearrange("k p t -> p k t"))
        nc.gpsimd.dma_start(out=wpT_sb[:], in_=wpT.rearrange("k p t -> p k t"))
        nc.gpsimd.dma_start(out=vaT_sb[:], in_=vaT[:])
        nc.gpsimd.dma_start(out=gc_sb[:], in_=gcw[:])
        nc.gpsimd.dma_start(out=bp_sb[:], in_=bpw[:])
        nc.gpsimd.dma_start(out=id16_sb[:], in_=id16[:])
        nc.gpsimd.dma_start(out=i16b_sb[:], in_=i16b[:])
        for b in range(BL):
            nc.gpsimd.dma_start(out=enct_sb[:, b, :, :],
                              in_=enct[b].rearrange("k p t -> p k t"))
        nc.vector.memset(hsT_sb[:], 0)
        nc.vector.memset(cs_sb[:], 0)

        # PSUM pools (8 banks total): big 2 + g 2 + qT 1 + tr 2 = 7
        ps_big = ctx.enter_context(tc.tile_pool(name="ps_big", bufs=2, space="PSUM"))
        ps_g = ctx.enter_context(tc.tile_pool(name="ps_g", bufs=2, space="PSUM"))
        ps_q = ctx.enter_context(tc.tile_pool(name="ps_q", bufs=1, space="PSUM"))
        ps_tr = ctx.enter_context(tc.tile_pool(name="ps_tr", bufs=2, space="PSUM"))

        # --- pre-loop: U[b] = Ua @ enc[b]^T ---
        with tc.tile_pool(name="preloop", bufs=2) as prepool:
            uaT_sb = prepool.tile([128, HC, H], BF16, tag="uaw")
            nc.gpsimd.dma_start(out=uaT_sb[:], in_=uaT.rearrange("k p t -> p k t"))
            for b in range(BL):
                est = prepool.tile([128, HC, T], BF16, tag="est")
                nc.gpsimd.dma_start(out=est[:], in_=encT[b].rearrange("k p t -> p k t"))
                for mc in range(HC):
                    pu = ps_big.tile([128, T], F32, tag="big")
                    for kc in range(HC):
                        nc.tensor.matmul(
                            pu[:], uaT_sb[:, kc, mc * 128:(mc + 1) * 128],
                            est[:, kc, :], start=(kc == 0), stop=(kc == HC - 1))
                    nc.vector.tensor_copy(U_sb[:, b, mc, :], pu[:])

        tc.strict_bb_all_engine_barrier()
        work = ctx.enter_context(tc.tile_pool(name="work", bufs=2))
        work1 = ctx.enter_context(tc.tile_pool(name="work1", bufs=1))
        xpool = ctx.enter_context(tc.tile_pool(name="xpool", bufs=2))

        from contextlib import contextmanager

        @contextmanager
        def loop_ctx():
            n = bench_steps or out_len
            if unroll:
                yield list(range(out_len))
            elif n % 2 == 0:
                with tc.For_i(0, n, 2,
                              hint_engines=(mybir.EngineType.PE,)) as i:
                    yield [i, i + 1]
            else:
                with tc.For_i(0, n, 1,
                              hint_engines=(mybir.EngineType.PE,)) as i:
                    yield [i]

        with loop_ctx() as ivs:
          for iv in ivs:
              # ---- qT[g, b] = 0.5*Wa @ 2h ----
              qT_ps = ps_q.tile([128, HC, BL], F32, tag="qT")
              for mc in range(HC):
                  for kc in range(HC):
                      nc.tensor.matmul(
                          qT_ps[:, mc, :], waT_sb[:, kc, mc * 128:(mc + 1) * 128],
                          hsT_sb[:, kc, :], start=(kc == 0), stop=(kc == HC - 1))
              qT_sb = work.tile([128, HC, BL], F32, tag="qT_sb")
              nc.vector.tensor_copy(qT_sb[:], qT_ps[:])

              # ---- scores ----
              scores_sb = work.tile([BL, T], F32, tag="scores")
              for bg in range(4):
                  sc_ps = ps_big.tile([128, T], F32, tag="big")
                  for j in range(4):
                      b = bg * 4 + j
                      xb = xpool.tile([128, HC, T], BF16, tag="X", name=f"X{b}")
                      for hc in range(HC):
                          nc.scalar.activation(
                              out=xb[:, hc, :], in_=U_sb[:, b, hc, :],
                              func=AF.Tanh, bias=qT_sb[:, hc, b:b + 1], scale=1.0)
                      for hc in range(HC):
                          nc.tensor.matmul(
                              sc_ps[32 * j:32 * j + 1, :], vaT_sb[:, hc:hc + 1],
                              xb[:, hc, :], start=(hc == 0), stop=(hc == HC - 1),
                              tile_position=(0, 32 * j))
                  st = work.tile([128, T], F32, tag="stage", name=f"stsc{bg}")
                  nc.vector.tensor_copy(st[:], sc_ps[:])
                  nc.sync.dma_start(
                      out=scores_sb[bg * 4:(bg + 1) * 4, :],
                      in_=st.rearrange("(j k) t -> j k t", j=4)[:, 0, :])

              # ---- softmax (in place on scores_sb) ----
              mneg = work1.tile([BL, 1], F32, tag="mneg")
              nc.vector.tensor_reduce(mneg[:], scores_sb[:], axis=AX.X, op=ALU.max,
                                      negate=True)
              esc = scores_sb
              nc.scalar.activation(out=esc[:], in_=scores_sb[:], func=AF.Exp,
                                   bias=mneg[:], scale=1.0)
              ssum = work1.tile([BL, 1], F32, tag="ssum")
              nc.vector.tensor_reduce(ssum[:], esc[:], axis=AX.X, op=ALU.add)
              nc.vector.reciprocal(ssum[:], ssum[:])
              nc.vector.tensor_scalar(esc[:], esc[:], ssum[:], None, ALU.mult)

              # ---- wT, ctx ----
              wt_ps = ps_tr.tile([128, TC, BL], F32, tag="tr")
              for t_c in range(TC):
                  nc.tensor.transpose(wt_ps[:, t_c, :],
                                      esc[:, t_c * 128:(t_c + 1) * 128], id16_sb[:])
              wt_sb = work.tile([128, TC, BL], BF16, tag="wt_sb")
              nc.vector.tensor_copy(wt_sb[:], wt_ps[:])

              ctx_sb = work.tile([BL, H], F32, tag="scores", name="ctx_sb")
              for bg in range(4):
                  cx_ps = ps_big.tile([128, H], F32, tag="big")
                  for t_c in range(TC):
                      for j in range(4):
                          b = bg * 4 + j
                          nc.tensor.matmul(
                              cx_ps[32 * j:32 * j + 1, :], wt_sb[:, t_c, b:b + 1],
                              enct_sb[:, b, t_c, :], start=(t_c == 0),
                              stop=(t_c == TC - 1), tile_position=(0, 32 * j))
                  st = work.tile([128, H], F32, tag="stage", name=f"stcx{bg}")
                  nc.vector.tensor_copy(st[:], cx_ps[:])
                  nc.sync.dma_start(
                      out=ctx_sb[bg * 4:(bg + 1) * 4, :],
                      in_=st.rearrange("(j k) t -> j k t", j=4)[:, 0, :])

              ct_ps = ps_tr.tile([128, HC, BL], F32, tag="tr")
              for hc in range(HC):
                  nc.tensor.transpose(ct_ps[:, hc, :],
                                      ctx_sb[:, hc * 128:(hc + 1) * 128], id16_sb[:])
              ctxT_sb = work.tile([128, HC, BL], BF16, tag="ctxT_sb")
              nc.vector.tensor_copy(ctxT_sb[:], ct_ps[:])

              # ---- gates + LSTM ----
              tg4 = []
              for gi in range(4):  # i, f, g, o chunks of 512
                  g_ps = ps_g.tile([BL, H], F32, tag="g")
                  nc.tensor.matmul(g_ps[:], i16b_sb[:],
                                   gc_sb[:, gi * H:(gi + 1) * H], start=True,
                                   stop=False, skip_group_check=True)
                  for kc in range(HC):
                      nc.tensor.matmul(g_ps[:], ctxT_sb[:, kc, :],
                                       wctxT_sb[:, kc, gi * H:(gi + 1) * H],
                                       start=False, stop=False,
                                       skip_group_check=True)
                  for kc in range(HC):
                      nc.tensor.matmul(g_ps[:], hsT_sb[:, kc, :],
                                       whhT_sb[:, kc, gi * H:(gi + 1) * H],
                                       start=False, stop=(kc == HC - 1),
                                       skip_group_check=True)
                  tg = work1.tile([BL, H], F32, tag=f"tg{gi}")
                  nc.scalar.activation(out=tg[:], in_=g_ps[:], func=AF.Tanh,
                                       bias=0.0, scale=(1.0 if gi == 2 else 0.5))
                  tg4.append(tg)
              ti, tf, tgg, to = tg4
              # 2*c_new = c + tf*c + tg + ti*tg
              nc.vector.tensor_mul(tf[:], tf[:], cs_sb[:])
              nc.vector.tensor_add(tf[:], tf[:], cs_sb[:])
              nc.vector.tensor_mul(ti[:], ti[:], tgg[:])
              nc.vector.tensor_add(ti[:], ti[:], tgg[:])
              nc.vector.tensor_add(ti[:], ti[:], tf[:])  # ti = 2*c_new
              tcell = work1.tile([BL, H], F32, tag="tcell")
              nc.scalar.activation(out=tcell[:], in_=ti[:], func=AF.Tanh,
                                   bias=0.0, scale=0.5)
              nc.vector.tensor_scalar(cs_sb[:], ti[:], 0.5, None, ALU.mult)
              # 2h = tcell + to*tcell
              nc.vector.tensor_mul(to[:], to[:], tcell[:])
              nc.vector.tensor_add(to[:], to[:], tcell[:])

              ht_ps = ps_tr.tile([128, HC, BL], F32, tag="tr")
              for hc in range(HC):
                  nc.tensor.transpose(ht_ps[:, hc, :],
                                      to[:, hc * 128:(hc + 1) * 128], id16_sb[:])
              nc.vector.tensor_copy(hsT_sb[:], ht_ps[:])

              # ---- y ----
              y_ps = ps_tr.tile([128, BL], F32, tag="tr")
              for kc in range(HC):
                  nc.tensor.matmul(y_ps[:], wpT_sb[:, kc, :], hsT_sb[:, kc, :],
                                   start=(kc == 0), stop=(kc == HC - 1))
              y_sb = work.tile([128, BL], F32, tag="y_sb")
              nc.vector.tensor_scalar(y_sb[:], y_ps[:], bp_sb[:], None, ALU.add)
              if bench_steps:
                  nc.sync.dma_start(out=yT[0:1], in_=y_sb[:])
              else:
                  nc.sync.dma_start(out=yT[bass.ts(iv, 1)], in_=y_sb[:])

    nc.finalize()
    return nc


_CACHE = {}


def _get_nc(out_len):
    if out_len not in _CACHE:
        _CACHE[out_len] = build(out_len)
    return _CACHE[out_len]


def kernel(encoder_outputs, latent_h, Wa, Ua, Va, W_ih, W_hh, b_ih, b_hh, Wp, bp,
           out_len):
    out_len = int(out_len)
    bf = ml_dtypes.bfloat16
    enc = np.asarray(encoder_outputs, np.float32)
    latent = np.asarray(latent_h, np.float32)
    Wa = np.asarray(Wa, np.float32)
    Ua = np.asarray(Ua, np.float32)
    Va = np.asarray(Va, np.float32)
    W_ih = np.asarray(W_ih, np.float32)
    W_hh = np.asarray(W_hh, np.float32)
    b_ih = np.asarray(b_ih, np.float32)
    b_hh = np.asarray(b_hh, np.float32)
    Wp = np.asarray(Wp, np.float32)
    bp = np.asarray(bp, np.float32)

    encT_a = np.ascontiguousarray(
        enc.transpose(0, 2, 1).reshape(B, HC, 128, T)).astype(bf)
    enct_a = np.ascontiguousarray(enc.reshape(B, TC, 128, H)).astype(bf)
    uaT_a = np.ascontiguousarray(Ua.T.reshape(HC, 128, H)).astype(bf)
    waT_a = np.ascontiguousarray((0.5 * Wa.T).reshape(HC, 128, H)).astype(bf)
    wctxT_a = np.ascontiguousarray(W_ih[:, H:].T.reshape(HC, 128, G4)).astype(bf)
    whhT_a = np.ascontiguousarray((0.5 * W_hh.T).reshape(HC, 128, G4)).astype(bf)
    wpT_a = np.ascontiguousarray((0.5 * Wp.T).reshape(HC, 128, D)).astype(bf)
    vaT_a = np.ascontiguousarray(Va[0].reshape(HC, 128).T).astype(bf)
    gc_a = (latent @ W_ih[:, :H].T + b_ih + b_hh).astype(bf)  # (B, 4H)
    bp_a = bp.reshape(128, 1).astype(np.float32)
    id16_a = np.eye(16, dtype=np.float32)
    i16b_a = np.eye(16).astype(bf)

    nc = _get_nc(out_len)
    in_maps = []
    for c in range(NCORES):
        s = slice(c * BL, (c + 1) * BL)
        in_maps.append({
            "encT": encT_a[s], "enct": enct_a[s], "uaT": uaT_a, "waT": waT_a,
            "wctxT": wctxT_a, "whhT": whhT_a, "wpT": wpT_a, "vaT": vaT_a,
            "gcw": gc_a[s], "bpw": bp_a, "id16": id16_a, "i16b": i16b_a,
        })
    import os
    trace = bool(os.environ.get("KERNEL_TRACE"))
    res = run_bass_kernel_spmd(nc, in_maps, core_ids=list(range(NCORES)),
                               trace=trace)
    if res.exec_time_ns is not None:
        print(f"HW exec time: {res.exec_time_ns} ns", flush=True)
        if res.instructions_and_trace is not None:
            print(f"trace: {res.instructions_and_trace[1]}", flush=True)
    ys = [r["yT"].transpose(2, 0, 1) for r in res.results]  # (BL, out_len, D)
    return np.concatenate(ys, axis=0).astype(np.float32)

